# revision 5
# baseline (speedup 1.0000x reference)
"""Trainium2 Bass kernel for nn_Knowledge_Decomposition.

Computation (per reference):
  g_spec = MLP_gs(gfeat);  p_spec = MLP_ps(pfeat)
  common = Interaction(a=pfeat, b=gfeat; c_* params)
  synergy = Interaction(a=pfeat, b=gfeat; s_* params)
where MLP(x) = relu(LN(x @ W.T + b) * g + beta) and Interaction computes
  g_align = MLP_g(a), p_align = MLP_p(b)
  out = p_align * sigmoid(p_align * <g_align, awp> + abp)
      + g_align * sigmoid(g_align * <p_align, awg> + abg)

Sharding: pure data parallel. B=128 rows split across 8 cores (16 rows,
i.e. 256 tokens of dim 256 per core); params replicated.

Layout on core: tokens on SBUF partitions (2 chunks of 128), features on
the free dim. x is PE-transposed once per core so the contraction dim
feeds the matmul partitions; weights are pre-transposed on the host.
"""

import sys

if "/opt/trn_rl_repo" not in sys.path:
    sys.path.insert(0, "/opt/trn_rl_repo")

import numpy as np

import concourse.bacc as bacc
import concourse.bass as bass
from concourse import mybir
from concourse.masks import make_identity
from concourse.tile import TileContext
from concourse.bass_utils import run_bass_kernel_spmd

AF = mybir.ActivationFunctionType
ALU = mybir.AluOpType
F32 = mybir.dt.float32

N_CORES = 8
B, L, D = 128, 16, 256
BS = B // N_CORES          # batch rows per core
T = BS * L                 # tokens per core = 256
P = 128                    # SBUF partitions
NT = T // P                # token chunks per core = 2
NK = D // P                # contraction chunks = 2
LN_EPS = 1e-5

MLPS = ["gs", "ps", "c_g", "c_p", "s_g", "s_p"]
# which transposed input feeds each MLP ('g' = gfeat, 'p' = pfeat).
# NOTE: reference calls interaction(a=pfeat, bfeat=gfeat): the *_g MLPs
# (g_align) consume pfeat and the *_p MLPs (p_align) consume gfeat.
MLP_INPUT = {"gs": "g", "ps": "p", "c_g": "p", "c_p": "g", "s_g": "p", "s_p": "g"}
MLP_BY_INP = {"g": ["gs", "c_p", "s_p"], "p": ["ps", "c_g", "s_g"]}
# LN processing order: interaction inputs first so interactions start early
LN_ORDER = ["c_g", "c_p", "s_g", "s_p", "gs", "ps"]
AW_KEYS = ["c_g", "c_p", "s_g", "s_p"]  # c_g<-c_agw, c_p<-c_apw, ...


def _bcast_rows(ap, p):
    """Broadcast a [N] DRAM AP across p partitions -> [p, N] (stride-0)."""
    return bass.AP(tensor=ap.tensor, offset=ap.offset, ap=[[0, p]] + list(ap.ap))


def _build(affine_identity: bool, ab: dict[str, float]):
    """Build + compile the per-core Bass program (SPMD; same on all cores)."""
    nc = bacc.Bacc("TRN2", target_bir_lowering=False, debug=False)

    xg = nc.dram_tensor("xg", [T, D], F32, kind="ExternalInput")
    xp = nc.dram_tensor("xp", [T, D], F32, kind="ExternalInput")
    xin = {"g": xg, "p": xp}
    wt_d = {m: nc.dram_tensor(f"wt_{m}", [D, D], F32, kind="ExternalInput") for m in MLPS}
    aw_d = {k: nc.dram_tensor(f"aw_{k}", [D], F32, kind="ExternalInput") for k in AW_KEYS}
    if not affine_identity:
        b_d = {m: nc.dram_tensor(f"b_{m}", [D], F32, kind="ExternalInput") for m in MLPS}
        g_d = {m: nc.dram_tensor(f"g_{m}", [D], F32, kind="ExternalInput") for m in MLPS}
        bt_d = {m: nc.dram_tensor(f"bt_{m}", [D], F32, kind="ExternalInput") for m in MLPS}
    outs = {
        name: nc.dram_tensor(name, [T, D], F32, kind="ExternalOutput")
        for name in ["o_common", "o_synergy", "o_gspec", "o_pspec"]
    }

    with TileContext(nc) as tc:
        with (
            tc.tile_pool(name="consts", bufs=1) as consts,
            tc.tile_pool(name="xnat", bufs=4) as xnat,
            tc.tile_pool(name="work", bufs=14) as work,
            tc.tile_pool(name="spool", bufs=14) as spool,
            tc.tile_pool(name="tpsum", bufs=2, space="PSUM") as tpsum,
            tc.tile_pool(name="hpsum", bufs=6, space="PSUM") as hpsum,
        ):
            ident = consts.tile([P, P], F32)
            make_identity(nc, ident)
            eps_t = consts.tile([P, 1], F32)
            nc.vector.memset(eps_t[:], LN_EPS)
            abt = {}
            for k in AW_KEYS:
                abt[k] = consts.tile([P, 1], F32, tag=f"ab_{k}", name=f"ab_{k}")
                nc.vector.memset(abt[k][:], ab[k])

            # weights, pre-transposed on host: wt[k, j] = W[j, k]
            wt_t = {}
            for m in MLPS:
                wt_t[m] = consts.tile([P, NK, D], F32, tag=f"wt_{m}", name=f"wt_{m}")
                nc.sync.dma_start(
                    out=wt_t[m][:],
                    in_=wt_d[m][:].rearrange("(kb p) j -> p kb j", p=P),
                )
            # attention weight vectors broadcast across partitions
            awbc = {}
            for k in AW_KEYS:
                awbc[k] = consts.tile([P, D], F32, tag=f"aw_{k}", name=f"aw_{k}")
                nc.gpsimd.dma_start(out=awbc[k][:], in_=_bcast_rows(aw_d[k][:], P))

            if not affine_identity:
                ones_t = consts.tile([1, P], F32, tag="ones")
                nc.vector.memset(ones_t[:], 1.0)
                b_t, gbc, btbc = {}, {}, {}
                for m in MLPS:
                    b_t[m] = consts.tile([1, D], F32, tag=f"b_{m}", name=f"b_{m}")
                    nc.sync.dma_start(out=b_t[m][:], in_=b_d[m][:].rearrange("d -> 1 d"))
                    gbc[m] = consts.tile([P, D], F32, tag=f"g_{m}", name=f"g_{m}")
                    nc.gpsimd.dma_start(out=gbc[m][:], in_=_bcast_rows(g_d[m][:], P))
                    btbc[m] = consts.tile([P, D], F32, tag=f"bt_{m}", name=f"bt_{m}")
                    nc.gpsimd.dma_start(out=btbc[m][:], in_=_bcast_rows(bt_d[m][:], P))

            # x loaded naturally ([tok, feat]) then PE-transposed into
            # xt[inp][:, kb, t] = x[t, kb*P + p]  (feature chunks on partitions)
            xt = {}
            for inp in ("g", "p"):
                xt[inp] = consts.tile([P, NK, T], F32, tag=f"xt_{inp}", name=f"xt_{inp}")
                for nb in range(NT):
                    xn = xnat.tile([P, D], F32, tag="xn")
                    nc.sync.dma_start(out=xn[:], in_=xin[inp][nb * P:(nb + 1) * P, :])
                    for kb in range(NK):
                        tp = tpsum.tile([P, P], F32, tag="tp")
                        nc.tensor.transpose(tp[:], xn[:, kb * P:(kb + 1) * P], ident[:])
                        nc.scalar.copy(out=xt[inp][:, kb, nb * P:(nb + 1) * P], in_=tp[:])

            for nb in range(NT):
                tok = slice(nb * P, (nb + 1) * P)
                # ---- matmuls: h[m] = x_inp @ W_m.T (+ b_m) ----
                hp = {}
                for inp in ("g", "p"):
                    for kb in range(NK):
                        for m in MLP_BY_INP[inp]:
                            if kb == 0:
                                hp[m] = hpsum.tile([P, D], F32, tag="hp", name=f"hp_{m}")
                            nc.tensor.matmul(
                                hp[m][:],
                                lhsT=xt[inp][:, kb, tok],
                                rhs=wt_t[m][:, kb, :],
                                start=(kb == 0),
                                stop=(kb == NK - 1 and affine_identity),
                            )
                    if not affine_identity:
                        for m in MLP_BY_INP[inp]:
                            nc.tensor.matmul(
                                hp[m][:],
                                lhsT=ones_t[0:1, :],
                                rhs=b_t[m][0:1, :],
                                start=False,
                                stop=True,
                            )

                # ---- LayerNorm + relu ----
                aligns = {}
                for m in LN_ORDER:
                    stats = spool.tile([P, 6], F32, tag="stats")
                    nc.vector.bn_stats(stats[:], hp[m][:])
                    mv = spool.tile([P, 2], F32, tag="mv")
                    nc.vector.bn_aggr(mv[:], stats[:])
                    std = spool.tile([P, 1], F32, tag="std")
                    nc.scalar.activation(std[:], mv[:, 1:2], AF.Sqrt, bias=eps_t[:])
                    rstd = spool.tile([P, 1], F32, tag="rstd")
                    nc.vector.reciprocal(rstd[:], std[:])
                    # nmr = -mean * rstd  -> activation computes (h*rstd + nmr)
                    nmr = spool.tile([P, 1], F32, tag="nmr")
                    nc.vector.tensor_scalar(
                        nmr[:], mv[:, 0:1], scalar1=rstd[:], scalar2=-1.0,
                        op0=ALU.mult, op1=ALU.mult,
                    )
                    is_align = m not in ("gs", "ps")
                    otag = "align" if is_align else "spec"
                    ot = work.tile([P, D], F32, tag=otag)
                    if affine_identity:
                        nc.scalar.activation(ot[:], hp[m][:], AF.Relu, bias=nmr[:], scale=rstd[:])
                    else:
                        nc.scalar.activation(ot[:], hp[m][:], AF.Identity, bias=nmr[:], scale=rstd[:])
                        nc.vector.tensor_mul(ot[:], ot[:], gbc[m][:])
                        nc.vector.tensor_add(ot[:], ot[:], btbc[m][:])
                        nc.scalar.activation(ot[:], ot[:], AF.Relu)
                    if m == "gs":
                        nc.sync.dma_start(out=outs["o_gspec"][tok, :], in_=ot[:])
                    elif m == "ps":
                        nc.sync.dma_start(out=outs["o_pspec"][tok, :], in_=ot[:])
                    else:
                        aligns[m] = ot

                # ---- interactions ----
                for pr, oname in (("c", "o_common"), ("s", "o_synergy")):
                    gal = aligns[pr + "_g"]
                    pal = aligns[pr + "_p"]
                    # dp = <p_align, awg>, dg = <g_align, awp> (per token)
                    sc1 = work.tile([P, D], F32, tag="ttscratch")
                    dp = spool.tile([P, 1], F32, tag="dp")
                    nc.vector.tensor_mul(sc1[:], pal[:], awbc[pr + "_g"][:])
                    nc.vector.tensor_reduce(dp[:], sc1[:], axis=mybir.AxisListType.X, op=ALU.add)
                    sc2 = work.tile([P, D], F32, tag="ttscratch")
                    dg = spool.tile([P, 1], F32, tag="dg")
                    nc.vector.tensor_mul(sc2[:], gal[:], awbc[pr + "_p"][:])
                    nc.vector.tensor_reduce(dg[:], sc2[:], axis=mybir.AxisListType.X, op=ALU.add)
                    gat = work.tile([P, D], F32, tag="att")
                    nc.scalar.activation(gat[:], gal[:], AF.Sigmoid, bias=abt[pr + "_g"][:], scale=dp[:])
                    pat = work.tile([P, D], F32, tag="att")
                    nc.scalar.activation(pat[:], pal[:], AF.Sigmoid, bias=abt[pr + "_p"][:], scale=dg[:])
                    t1 = work.tile([P, D], F32, tag="t1")
                    nc.vector.tensor_mul(t1[:], pal[:], pat[:])
                    t2 = work.tile([P, D], F32, tag="t2")
                    nc.gpsimd.tensor_mul(t2[:], gal[:], gat[:])
                    ot2 = work.tile([P, D], F32, tag="iout")
                    nc.vector.tensor_add(ot2[:], t1[:], t2[:])
                    nc.sync.dma_start(out=outs[oname][tok, :], in_=ot2[:])

    nc.compile()
    return nc


_CACHE: dict = {}


def _get_program(affine_identity: bool, ab: dict[str, float]):
    key = (affine_identity, tuple(sorted(ab.items())))
    if key not in _CACHE:
        _CACHE[key] = _build(affine_identity, ab)
    return _CACHE[key]


def kernel(**inputs) -> tuple:
    inp = {k: np.asarray(v) for k, v in inputs.items()}
    gfeat = np.ascontiguousarray(inp["gfeat"], dtype=np.float32)
    pfeat = np.ascontiguousarray(inp["pfeat"], dtype=np.float32)

    affine_identity = all(
        (inp[m + "_b"] == 0).all()
        and (inp[m + "_g"] == 1).all()
        and (inp[m + "_beta"] == 0).all()
        for m in MLPS
    )
    ab = {
        "c_g": float(inp["c_agb"]),
        "c_p": float(inp["c_apb"]),
        "s_g": float(inp["s_agb"]),
        "s_p": float(inp["s_apb"]),
    }
    nc = _get_program(affine_identity, ab)

    base = {
        f"wt_{m}": np.ascontiguousarray(inp[f"{m}_W"].T, dtype=np.float32)
        for m in MLPS
    }
    base["aw_c_g"] = np.ascontiguousarray(inp["c_agw"], dtype=np.float32)
    base["aw_c_p"] = np.ascontiguousarray(inp["c_apw"], dtype=np.float32)
    base["aw_s_g"] = np.ascontiguousarray(inp["s_agw"], dtype=np.float32)
    base["aw_s_p"] = np.ascontiguousarray(inp["s_apw"], dtype=np.float32)
    if not affine_identity:
        for m in MLPS:
            base[f"b_{m}"] = np.ascontiguousarray(inp[f"{m}_b"], dtype=np.float32)
            base[f"g_{m}"] = np.ascontiguousarray(inp[f"{m}_g"], dtype=np.float32)
            base[f"bt_{m}"] = np.ascontiguousarray(inp[f"{m}_beta"], dtype=np.float32)

    gsh = gfeat.reshape(N_CORES, T, D)
    psh = pfeat.reshape(N_CORES, T, D)
    in_maps = [dict(base, xg=gsh[c], xp=psh[c]) for c in range(N_CORES)]

    res = run_bass_kernel_spmd(nc, in_maps, list(range(N_CORES)))

    def gather(name):
        return np.concatenate(
            [res.results[c][name].reshape(BS, L, D) for c in range(N_CORES)], axis=0
        )

    return (gather("o_common"), gather("o_synergy"), gather("o_gspec"), gather("o_pspec"))
